# revision 12
# baseline (speedup 1.0000x reference)
"""Trainium2 Bass kernel for nn_Conv_89713276879316.

Reference semantics (faithful channel bug): take ONLY the last channel of
image [32, 3, 512, 512], zero-pad by 7, cross-correlate with the 15x15
kernel, broadcast the [32, 1, 512, 512] result to all 3 channels.

Strategy:
  - Host: extract channel 2, zero-pad to [32, 526, 526] fp32.
  - Device (per core, 4 images): 15x15 conv as banded matmuls on TensorE
    in float32r (full-rate fp32 matmul mode, ~1e-4 rel err; fp32 PSUM
    accumulation). For each 114-row output tile, 15 matmuls (one per
    kernel column dx) contract 128 padded input rows against a banded
    matrix B_dx[r, y] = w[r - y, dx]; dx is a free-dim offset into the
    padded image rows; all 15 accumulate into one PSUM bank. Two tiles'
    matmul streams are interleaved so the PE never idles at PSUM-bank
    (accumulation-group) boundaries; 8 PSUM banks cycle so the DVE
    PSUM->SBUF copies and output DMAs stay off the critical path.
  - Host: gather 8 shards, broadcast channel dim to 3.

Measured (8 axon-tunneled trn2 cores, dispatch-floor-free loop timing):
~78 us device time, rel err 1.1e-4. Alternatives wired via MODE:
"f16x2" (fp16 hi/lo 2-pass) ~160 us at 2.2e-7 err; "f32" exact ~4x slower.
"""

import sys

import numpy as np

try:
    import concourse.bass as bass
except ImportError:  # pragma: no cover - fallback path inside the container
    sys.path.insert(0, "/opt/trn_rl_repo")
    import concourse.bass as bass

import ml_dtypes
from contextlib import ExitStack

import concourse.tile as tile
from concourse import bacc, mybir
from concourse.bass_utils import run_bass_kernel_spmd

N_CORES = 8
N_IMG = 32
C_IMG = 3
H = W = 512
KS = 15
PAD = KS // 2  # 7
HP = H + 2 * PAD  # 526
PER_CORE = N_IMG // N_CORES  # 4
MTILE = 114  # output rows per matmul tile (contract = MTILE + 14 <= 128)
TILES = []
_y = 0
while _y < H:
    TILES.append((_y, min(MTILE, H - _y)))
    _y += MTILE

F32 = mybir.dt.float32

# mode: "f16x2" = fp16 hi/lo 2-pass (~1e-6), "f16" = single pass (~2e-4),
#       "f32r" = single pass float32r (~1e-4), "f32" = exact 4x slower
MODE = "f32r"
# tunables
INTERLEAVE = 2   # output tiles whose matmul streams interleave
PSUM_BUFS = 8
IMG_BUFS_PER_PLANE = 4
OUT_BUFS = 4
COPY_ENGINE = "dve"  # "dve" | "act" | "alt"

_MODE_CFG = {
    "f16x2": (mybir.dt.float16, np.float16, 2),
    "f16": (mybir.dt.float16, np.float16, 1),
    "bf16x2": (mybir.dt.bfloat16, ml_dtypes.bfloat16, 2),
    "f32r": (mybir.dt.float32r, np.float32, 1),
    "f32": (mybir.dt.float32, np.float32, 1),
}

_CACHE = {}


def _build_nc(repeat=1, mode=None, loop=False):
    """Build the per-core Bass program (identical on all 8 cores).

    repeat > 1 re-runs the whole compute (same inputs/outputs) for
    dispatch-floor-free device timing; with loop=True the repetition is a
    Tile For_i loop instead of unrolling.
    """
    mode = mode or MODE
    mdt, _npdt, passes = _MODE_CFG[mode]
    nc = bacc.Bacc("TRN2", target_bir_lowering=False, debug=False)

    imgs = [
        nc.dram_tensor(f"img{p}", [PER_CORE, HP, HP], mdt, kind="ExternalInput").ap()
        for p in range(passes)
    ]
    bands = nc.dram_tensor("bands", [128, KS * MTILE], mdt, kind="ExternalInput").ap()
    out = nc.dram_tensor("out", [PER_CORE, H, W], F32, kind="ExternalOutput").ap()

    with tile.TileContext(nc) as tc, ExitStack() as ctx:
        bands_pool = ctx.enter_context(tc.tile_pool(name="bands", bufs=1))
        img_pool = ctx.enter_context(
            tc.tile_pool(name="img", bufs=IMG_BUFS_PER_PLANE * passes)
        )
        psum_pool = ctx.enter_context(
            tc.tile_pool(name="psum", bufs=PSUM_BUFS, space="PSUM")
        )
        out_pool = ctx.enter_context(tc.tile_pool(name="outp", bufs=OUT_BUFS))

        bands_sb = bands_pool.tile([128, KS * MTILE], mdt)
        nc.sync.dma_start(bands_sb[:], bands[:, :])

        worklist = [(i, y0, m) for i in range(PER_CORE) for (y0, m) in TILES]
        n_mm = passes * KS
        cnt = 0

        def body(_iv=None):
            nonlocal cnt
            for g in range(0, len(worklist), INTERLEAVE):
                group = worklist[g : g + INTERLEAVE]
                units = []  # (i, y0, m, srcs, ps)
                for (i, y0, m) in group:
                    r = m + KS - 1  # input rows needed
                    srcs = []
                    for p in range(passes):
                        t = img_pool.tile([128, HP], mdt, name="imgt", tag=f"img{p}")
                        nc.sync.dma_start(t[:r, :], imgs[p][i, y0 : y0 + r, :])
                        srcs.append(t)
                    ps = psum_pool.tile([MTILE, W], F32, name="ps", tag="ps")
                    units.append((i, y0, m, srcs, ps))

                # interleave the matmul streams of the group's tiles so the
                # PE never idles at an accumulation-group boundary
                for k in range(n_mm):
                    p, dx = divmod(k, KS)
                    for (i, y0, m, srcs, ps) in units:
                        r = m + KS - 1
                        nc.tensor.matmul(
                            ps[:m, :],
                            bands_sb[:r, dx * MTILE : dx * MTILE + m],
                            srcs[p][:r, dx : dx + W],
                            start=(k == 0),
                            stop=(k == n_mm - 1),
                        )

                for (i, y0, m, srcs, ps) in units:
                    ot = out_pool.tile([MTILE, W], F32, name="ot", tag="ot")
                    eng = COPY_ENGINE
                    if eng == "alt":
                        eng = "dve" if cnt % 2 == 0 else "act"
                    if eng == "dve":
                        nc.vector.tensor_copy(ot[:m, :], ps[:m, :])
                    else:
                        nc.scalar.copy(ot[:m, :], ps[:m, :])
                    cnt += 1
                    nc.sync.dma_start(out[i, y0 : y0 + m, :], ot[:m, :])

        if loop and repeat > 1:
            # unroll 8 bodies per For_i iteration so the ~2us back-edge
            # barrier and lost cross-iteration overlap amortize away
            chunk = 8 if repeat % 8 == 0 else 1
            with tc.For_i(0, repeat // chunk, 1):
                for _u in range(chunk):
                    body()
        else:
            for _rep in range(repeat):
                body()

    nc.compile()
    return nc


def _prep_inputs(image: np.ndarray, kernel: np.ndarray, mode=None):
    """Host-side prep: channel select, pad, hi/lo split, band matrices."""
    mode = mode or MODE
    _mdt, npdt, passes = _MODE_CFG[mode]
    ch = np.ascontiguousarray(image[:, -1, :, :]).astype(np.float32)  # [32,512,512]
    padded = np.zeros((N_IMG, HP, HP), np.float32)
    padded[:, PAD : PAD + H, PAD : PAD + W] = ch
    planes = []
    rem = padded
    for p in range(passes):
        q = rem.astype(npdt)
        planes.append(q)
        if p + 1 < passes:
            rem = rem - q.astype(np.float32)

    w = kernel.astype(np.float32)
    bands = np.zeros((128, KS, MTILE), np.float32)
    for mm in range(MTILE):
        bands[mm : mm + KS, :, mm] = w  # B_dx[r, m] = w[r - m, dx]
    bands_c = bands.reshape(128, KS * MTILE).astype(npdt)
    return planes, bands_c


def kernel(image: np.ndarray, kernel: np.ndarray) -> np.ndarray:
    planes, bands_c = _prep_inputs(image, kernel)

    key = ("nc", MODE)
    if key not in _CACHE:
        _CACHE[key] = _build_nc()
    nc = _CACHE[key]

    in_maps = []
    for c in range(N_CORES):
        s = slice(c * PER_CORE, (c + 1) * PER_CORE)
        m = {f"img{p}": planes[p][s] for p in range(len(planes))}
        m["bands"] = bands_c
        in_maps.append(m)

    res = run_bass_kernel_spmd(nc, in_maps, core_ids=list(range(N_CORES)))
    _CACHE["last_results"] = res

    full = np.concatenate([res.results[c]["out"] for c in range(N_CORES)], axis=0)
    out = np.broadcast_to(full[:, None, :, :], (N_IMG, C_IMG, H, W))
    return np.ascontiguousarray(out)
